# revision 8
# baseline (speedup 1.0000x reference)
"""GAT residual block (nn_GATResBlock) on 8 Trainium2 NeuronCores.

Strategy
--------
- Shard destination nodes (and their incoming edges) across the 8 cores;
  each core owns a contiguous range of 6250 dst nodes.
- Host-side graph preprocessing (sanctioned by the sharding hint): sort each
  core's edges by dst block (128 dsts per block), build padded per-block edge
  lists and int16 gather-index arrays.
- Algebraic folds: a_src = x @ (W.T @ att_src-expanded) so the attention
  logits come out of the same projection matmul; segment-softmax max-trick is
  dropped (logits are bounded, softmax is shift invariant) and the softmax is
  normalized at the *node* level: agg = (sum ex*xp[src]) / (sum ex), so no
  per-edge alpha is ever materialized.
- Device per core: one replicated projection pass builds a DRAM node table
  T1[row] = [xp | a_src] (bf16); dst blocks are processed in groups of ~5 with
  ONE dma_gather per (group, table-half) to amortize the ~2.5us SWDGE call
  overhead.  Per block, a one-hot (edge,dst) selection matrix built with
  iota/is_equal routes a_dst to edges (tensor transpose + tiny matmul, with
  the gathered a_src accumulated into the same PSUM via an identity matmul),
  the LeakyReLU+exp run on the scalar (ACT) engine, and a PSUM-accumulated
  matmul reduces weighted messages + softmax denominators in one pass.
  Epilogue divides, adds the skip projection and applies ELU.
- int16 gather indices only span 32768 rows, so the node table is gathered by
  two calls per group: rows [0, 32768) ("A") and [32768, ...) ("B"); the host
  splits each block's edge list accordingly.  Pad slots gather table row 0
  (a_src = -60 so ex ~ 1e-5) and carry dloc = 500 so the one-hot zeroes them.
"""

import sys
import types

sys.path.insert(0, "/opt/trn_rl_repo")

import numpy as np

try:
    import ml_dtypes

    BF16_NP = ml_dtypes.bfloat16
except Exception:  # pragma: no cover
    BF16_NP = None


# ---------------------------------------------------------------------------
# NTFF profile hook (missing antenv.axon_hooks in this image). Needed only
# when tracing; harmless otherwise.
def _install_ntff_hook():
    if "antenv.axon_hooks" in sys.modules:
        return
    try:
        hooks = types.ModuleType("antenv.axon_hooks")
        _h = [None]
        hooks.set_axon_ntff_profile_hook = lambda h: _h.__setitem__(0, h)
        hooks.get_axon_ntff_profile_hook = lambda: _h[0]
        sys.modules["antenv.axon_hooks"] = hooks
        import antenv

        antenv.axon_hooks = hooks
        from trn_agent_boot.trn_boot import _ntff_profile_via_ctypes

        hooks.set_axon_ntff_profile_hook(
            _ntff_profile_via_ctypes("/opt/axon/libaxon_pjrt.so")
        )
    except Exception:
        pass


_install_ntff_hook()

from concourse import bacc, bass, mybir, tile  # noqa: E402
from concourse.bass_utils import run_bass_kernel_spmd  # noqa: E402

F32 = mybir.dt.float32
I16 = mybir.dt.int16
ALU = mybir.AluOpType
ACTF = mybir.ActivationFunctionType

P = 128
NEG_SLOPE = 0.2
PAD_ASRC = -60.0
PAD_DLOC = 500.0
GROUP = 5            # dst blocks per dma_gather call


class Cfg:
    def __init__(self, N=50000, IN=128, H=4, C=32, E=800000, NC=8, SPLIT=32768,
                 TAs=None, TBs=None):
        self.N, self.IN, self.H, self.C, self.E, self.NC = N, IN, H, C, E, NC
        self.HC = H * C
        assert self.HC == 128 and IN == 128
        assert N % NC == 0
        self.NLOC = N // NC                      # owned dst nodes per core
        self.NBLK = (self.NLOC + P - 1) // P     # dst blocks per core
        self.NLOCP = self.NBLK * P               # padded local nodes
        self.SPLIT = SPLIT                       # int16 A/B table split
        nrows = 1 + N + 1                        # PAD_A + nodes + PAD_B
        self.NR = ((nrows + P - 1) // P) * P     # node-table rows (padded)
        assert self.NR - SPLIT <= 32768
        self.PAD_B = N + 1                       # table row of the B pad
        self.ROWW = 256                  # T1 bf16 cols: xp(128)+a_src(4)+pad
        self.TAs, self.TBs = TAs, TBs            # per-block edge tiles (A/B)
        if TAs is not None:
            self._finish()

    def _finish(self):
        self.TAtot = sum(self.TAs)
        self.TBtot = sum(self.TBs)
        self.Tmax = max(a + b for a, b in zip(self.TAs, self.TBs))
        # block groups (gather-call granularity)
        self.groups = [list(range(g, min(g + GROUP, self.NBLK)))
                       for g in range(0, self.NBLK, GROUP)]
        self.gTA = [sum(self.TAs[b] for b in g) for g in self.groups]
        self.gTB = [sum(self.TBs[b] for b in g) for g in self.groups]
        self.gTAmax = max(self.gTA)
        self.gTBmax = max(self.gTB)
        # tile offset of block b's A (resp B) region within its group tile
        self.aoff, self.boff = {}, {}
        for gi, g in enumerate(self.groups):
            oa, ob = 0, 0
            for b in g:
                self.aoff[b] = oa
                self.boff[b] = ob
                oa += self.TAs[b]
                ob += self.TBs[b]
        # column offset of block b's A/B tiles in the concatenated idx arrays
        self.acol, self.bcol = {}, {}
        oa = ob = 0
        for b in range(self.NBLK):
            self.acol[b], self.bcol[b] = oa, ob
            oa += self.TAs[b]
            ob += self.TBs[b]


# ---------------------------------------------------------------------------
# Host-side preprocessing: edge partitioning + gather index construction.


def _wrap_idx(arr):
    """[K*128] edge-slot array -> [128, K*8] int16 'wrapped' index layout
    (index i lives at [i % 16, i // 16], replicated across the 8 groups)."""
    k16 = arr.reshape(-1, 16).T.astype(np.int16)  # [16, K*8]
    return np.tile(k16, (8, 1))                   # [128, K*8]


def preprocess(cfg, edge_index):
    """Build per-core gather index arrays from the (2, E) edge list."""
    src = np.asarray(edge_index[0], dtype=np.int64)
    dst = np.asarray(edge_index[1], dtype=np.int64)
    core = dst // cfg.NLOC
    dstl = dst - core * cfg.NLOC
    blk = dstl // P
    srow = src + 1                                # +1: table row 0 is PAD_A
    isB = (srow >= cfg.SPLIT).astype(np.int64)

    order = np.lexsort((srow, isB, blk, core))
    core_s, blk_s, isB_s = core[order], blk[order], isB[order]
    srow_s, dstl_s = srow[order], dstl[order]

    gid = ((core_s * cfg.NBLK) + blk_s) * 2 + isB_s
    ngroups = cfg.NC * cfg.NBLK * 2
    counts = np.bincount(gid, minlength=ngroups)
    starts = np.concatenate(([0], np.cumsum(counts)[:-1]))
    rank = np.arange(len(gid)) - starts[gid]

    cA = counts.reshape(cfg.NC, cfg.NBLK, 2)[:, :, 0]   # [NC, NBLK]
    cB = counts.reshape(cfg.NC, cfg.NBLK, 2)[:, :, 1]
    if cfg.TAs is None:
        cfg.TAs = tuple(max(1, int(-(-cA[:, b].max() // P)))
                        for b in range(cfg.NBLK))
        cfg.TBs = tuple(max(1, int(-(-cB[:, b].max() // P)))
                        for b in range(cfg.NBLK))
        cfg._finish()
    for b in range(cfg.NBLK):
        assert cA[:, b].max() <= cfg.TAs[b] * P
        assert cB[:, b].max() <= cfg.TBs[b] * P

    # per (core, block) padded slot arrays
    idxA = {}
    idxB = {}
    dlocA = {}
    dlocB = {}
    for c in range(cfg.NC):
        for b in range(cfg.NBLK):
            idxA[c, b] = np.zeros(cfg.TAs[b] * P, dtype=np.int64)
            idxB[c, b] = np.full(cfg.TBs[b] * P, cfg.PAD_B - cfg.SPLIT,
                                 dtype=np.int64)
            dlocA[c, b] = np.full(cfg.TAs[b] * P, PAD_DLOC, dtype=np.float32)
            dlocB[c, b] = np.full(cfg.TBs[b] * P, PAD_DLOC, dtype=np.float32)

    a = isB_s == 0
    b_ = ~a
    # vectorized fill
    for c in range(cfg.NC):
        for b in range(cfg.NBLK):
            selA = a & (core_s == c) & (blk_s == b)
            idxA[c, b][rank[selA]] = srow_s[selA]
            dlocA[c, b][rank[selA]] = dstl_s[selA] - b * P
            selB = b_ & (core_s == c) & (blk_s == b)
            idxB[c, b][rank[selB]] = srow_s[selB] - cfg.SPLIT
            dlocB[c, b][rank[selB]] = dstl_s[selB] - b * P

    per_core = []
    for c in range(cfg.NC):
        wA = np.concatenate([_wrap_idx(idxA[c, b]) for b in range(cfg.NBLK)],
                            axis=1)
        wB = np.concatenate([_wrap_idx(idxB[c, b]) for b in range(cfg.NBLK)],
                            axis=1)
        # dloc layout [128, tiles]: [p, tile_col + t] = slot (t, p)
        dA = np.concatenate(
            [dlocA[c, b].reshape(cfg.TAs[b], P).T for b in range(cfg.NBLK)],
            axis=1)
        dB = np.concatenate(
            [dlocB[c, b].reshape(cfg.TBs[b], P).T for b in range(cfg.NBLK)],
            axis=1)
        per_core.append(dict(idxA=np.ascontiguousarray(wA),
                             idxB=np.ascontiguousarray(wB),
                             dlocA=np.ascontiguousarray(dA),
                             dlocB=np.ascontiguousarray(dB)))
    return per_core


def make_weights(cfg, W, att_src, att_dst, bias, skip_W, skip_b):
    """Fold attention vectors into the projection weights."""
    H, C, IN = cfg.H, cfg.C, cfg.IN
    A_s = np.zeros((IN, H), dtype=np.float32)
    A_d = np.zeros((IN, H), dtype=np.float32)
    for h in range(H):
        # a_src[n,h] = sum_c xp[n,h*C+c]*att_src[h,c] = x @ (W[h*C:+C].T @ att)
        A_s[:, h] = W[h * C:(h + 1) * C, :].T @ att_src[0, h]
        A_d[:, h] = W[h * C:(h + 1) * C, :].T @ att_dst[0, h]
    Wcat = np.concatenate([W.T, A_s, A_d], axis=1).astype(np.float32)  # [IN,136]
    Wsk = np.concatenate([skip_W.T, A_d], axis=1).astype(np.float32)   # [IN,132]
    bias2 = np.tile((bias + skip_b).astype(np.float32)[None, :], (P, 1))
    return Wcat, Wsk, bias2


def make_inputs(cfg, x, edge_index, W, att_src, att_dst, bias, skip_W, skip_b):
    per_core_idx = preprocess(cfg, edge_index)
    Wcat, Wsk, bias2 = make_weights(cfg, W, att_src, att_dst, bias, skip_W,
                                    skip_b)
    xT = np.zeros((cfg.IN, cfg.NR), dtype=np.float32)
    xT[:, 1:1 + cfg.N] = np.asarray(x, dtype=np.float32).T
    xT = xT.astype(BF16_NP)
    iota = np.tile(np.arange(P, dtype=np.float32)[None, :], (P, 1))
    iotap = np.tile(np.arange(P, dtype=np.float32)[:, None], (1, P))
    negr = np.full((1, 4), PAD_ASRC, dtype=np.float32)

    in_maps = []
    for c in range(cfg.NC):
        xTl = np.zeros((cfg.IN, cfg.NLOCP), dtype=np.float32)
        xTl[:, :cfg.NLOC] = np.asarray(
            x[c * cfg.NLOC:(c + 1) * cfg.NLOC], dtype=np.float32).T
        m = dict(xT=xT, xTl=np.ascontiguousarray(xTl.astype(BF16_NP)),
                 Wcat=Wcat, Wsk=Wsk,
                 bias2=bias2, iota=iota, iotap=iotap, negr=negr,
                 **per_core_idx[c])
        in_maps.append(m)
    return in_maps


# ---------------------------------------------------------------------------
# Device program.


def build_program(cfg, debug_level=99):
    """Build the per-core SPMD Bass program."""
    nc = bacc.Bacc(None)
    NBLK, NR, ROWW = cfg.NBLK, cfg.NR, cfg.ROWW
    BF16 = mybir.dt.bfloat16

    xT = nc.declare_dram_parameter("xT", [cfg.IN, NR], BF16, isOutput=False)
    xTl = nc.declare_dram_parameter("xTl", [cfg.IN, cfg.NLOCP], BF16,
                                    isOutput=False)
    Wcat = nc.declare_dram_parameter("Wcat", [cfg.IN, 136], F32, isOutput=False)
    Wsk = nc.declare_dram_parameter("Wsk", [cfg.IN, 132], F32, isOutput=False)
    bias2 = nc.declare_dram_parameter("bias2", [P, 128], F32, isOutput=False)
    iota = nc.declare_dram_parameter("iota", [P, P], F32, isOutput=False)
    iotap = nc.declare_dram_parameter("iotap", [P, P], F32, isOutput=False)
    negr = nc.declare_dram_parameter("negr", [1, 4], F32, isOutput=False)
    idxA = nc.declare_dram_parameter("idxA", [P, cfg.TAtot * 8], I16,
                                     isOutput=False)
    idxB = nc.declare_dram_parameter("idxB", [P, cfg.TBtot * 8], I16,
                                     isOutput=False)
    dlocA = nc.declare_dram_parameter("dlocA", [P, cfg.TAtot], F32,
                                      isOutput=False)
    dlocB = nc.declare_dram_parameter("dlocB", [P, cfg.TBtot], F32,
                                      isOutput=False)
    out = nc.declare_dram_parameter("out", [cfg.NLOCP, 128], F32,
                                    isOutput=True)

    T1 = nc.dram_tensor("T1", [NR, ROWW], BF16)

    with tile.TileContext(nc) as tc:
        with (
            tc.tile_pool(name="const", bufs=1) as cpool,
            tc.tile_pool(name="main", bufs=2) as mp,
            tc.tile_pool(name="sv", bufs=3) as svp,
            tc.tile_pool(name="epi", bufs=4) as ep,
        ):
            # ---- constants ----
            iota_sb = cpool.tile([P, P], F32)
            nc.sync.dma_start(out=iota_sb[:], in_=iota[:])
            iotap_sb = cpool.tile([P, P], F32)
            nc.sync.dma_start(out=iotap_sb[:], in_=iotap[:])
            ident_bf = cpool.tile([P, P], BF16)
            nc.vector.tensor_tensor(out=ident_bf[:], in0=iota_sb[:],
                                    in1=iotap_sb[:], op=ALU.is_equal)
            wcat_sb = cpool.tile([P, 136], F32)
            nc.sync.dma_start(out=wcat_sb[:], in_=Wcat[:])
            wcat_bf = cpool.tile([P, 136], BF16)
            nc.vector.tensor_copy(out=wcat_bf[:], in_=wcat_sb[:])
            wsk_sb = cpool.tile([P, 132], F32)
            nc.sync.dma_start(out=wsk_sb[:], in_=Wsk[:])
            wsk_bf = cpool.tile([P, 132], BF16)
            nc.vector.tensor_copy(out=wsk_bf[:], in_=wsk_sb[:])
            bias_sb = cpool.tile([P, 128], F32)
            nc.sync.dma_start(out=bias_sb[:], in_=bias2[:])
            negr_sb = cpool.tile([1, 4], F32)
            nc.sync.dma_start(out=negr_sb[:], in_=negr[:])
            idxA_sb = cpool.tile([P, cfg.TAtot * 8], I16)
            nc.sync.dma_start(out=idxA_sb[:], in_=idxA[:])
            idxB_sb = cpool.tile([P, cfg.TBtot * 8], I16)
            nc.sync.dma_start(out=idxB_sb[:], in_=idxB[:])
            dlocA_sb = cpool.tile([P, cfg.TAtot], F32)
            nc.sync.dma_start(out=dlocA_sb[:], in_=dlocA[:])
            dlocB_sb = cpool.tile([P, cfg.TBtot], F32)
            nc.sync.dma_start(out=dlocB_sb[:], in_=dlocB[:])
            skip_sb = cpool.tile([P, NBLK * 128], F32)
            adst_sb = cpool.tile([P, NBLK * 4], BF16)

            # ---- phase 1: global node table T1 = [xp | a_src] (bf16) ----
            with (
                tc.tile_pool(name="prol", bufs=3) as prol,
                tc.tile_pool(name="pp", bufs=2, space="PSUM") as pp,
            ):
                CH = 8
                for i0 in range(0, NR // P, CH):
                    ch = min(CH, NR // P - i0)
                    xt = prol.tile([P, CH * P], BF16)
                    nc.sync.dma_start(
                        out=xt[:, 0:ch * P], in_=xT[:, i0 * P:(i0 + ch) * P])
                    st4 = prol.tile([P, CH, 132], BF16)
                    for k in range(ch):
                        ps = pp.tile([P, 136], F32)
                        nc.tensor.matmul(out=ps[:],
                                         lhsT=xt[:, k * P:(k + 1) * P],
                                         rhs=wcat_bf[:], start=True, stop=True)
                        if k % 2 == 0:
                            nc.scalar.activation(out=st4[:, k, :],
                                                 in_=ps[:, 0:132],
                                                 func=ACTF.Copy)
                        else:
                            nc.vector.tensor_copy(out=st4[:, k, :],
                                                  in_=ps[:, 0:132])
                    nc.scalar.dma_start(
                        out=T1[i0 * P:(i0 + ch) * P, 0:132].rearrange(
                            "(k p) c -> p k c", p=P),
                        in_=st4[:, 0:ch, :])
                # pad rows: a_src = PAD_ASRC so padded edges contribute ~0
                negr_bf = cpool.tile([1, 4], BF16)
                nc.vector.tensor_copy(out=negr_bf[:], in_=negr_sb[:])
                nc.sync.dma_start(out=T1[0:1, 128:132], in_=negr_bf[:])
                nc.sync.dma_start(out=T1[cfg.PAD_B:cfg.PAD_B + 1, 128:132],
                                  in_=negr_bf[:])

                # ---- phase 2: local skip projection + a_dst (SBUF) ----
                for j in range(NBLK):
                    xl = prol.tile([P, P], BF16)
                    nc.sync.dma_start(out=xl[:], in_=xTl[:, j * P:(j + 1) * P])
                    ps2 = pp.tile([P, 132], F32, tag="ps2")
                    nc.tensor.matmul(out=ps2[:], lhsT=xl[:], rhs=wsk_bf[:],
                                     start=True, stop=True)
                    nc.vector.tensor_tensor(out=skip_sb[:, j * P:(j + 1) * P],
                                            in0=ps2[:, 0:128], in1=bias_sb[:],
                                            op=ALU.add)
                    nc.scalar.activation(out=adst_sb[:, j * 4:(j + 1) * 4],
                                         in_=ps2[:, 128:132], func=ACTF.Copy)

            # ---- phase 3: per-dst-block edge processing ----
            GT = cfg.gTAmax + cfg.gTBmax
            Tm = cfg.Tmax
            with (
                tc.tile_pool(name="acc", bufs=2, space="PSUM") as ap,
                tc.tile_pool(name="stp", bufs=2, space="PSUM") as sp,
                tc.tile_pool(name="adp", bufs=2, space="PSUM") as adp,
            ):
                for gi, grp in enumerate(cfg.groups):
                    gTA, gTB = cfg.gTA[gi], cfg.gTB[gi]
                    ac0 = cfg.acol[grp[0]]
                    bc0 = cfg.bcol[grp[0]]
                    G1 = mp.tile([P, GT, ROWW], BF16)
                    nc.gpsimd.dma_gather(
                        out_ap=G1[:, 0:gTA, :],
                        in_ap=T1[:],
                        idxs_ap=idxA_sb[:, ac0 * 8:(ac0 + gTA) * 8],
                        num_idxs=gTA * P,
                        num_idxs_reg=gTA * P,
                        elem_size=ROWW,
                        single_packet=False,
                    )
                    nc.gpsimd.dma_gather(
                        out_ap=G1[:, gTA:gTA + gTB, :],
                        in_ap=T1[cfg.SPLIT:, :],
                        idxs_ap=idxB_sb[:, bc0 * 8:(bc0 + gTB) * 8],
                        num_idxs=gTB * P,
                        num_idxs_reg=gTB * P,
                        elem_size=ROWW,
                        single_packet=False,
                    )

                    for b in grp:
                        TA_b, TB_b = cfg.TAs[b], cfg.TBs[b]
                        T_b = TA_b + TB_b
                        ao = cfg.aoff[b]              # tiles into G1 A region
                        bo = gTA + cfg.boff[b]        # tiles into G1
                        # block slot view: A tiles [ao, ao+TA_b),
                        #                  B tiles [bo, bo+TB_b)

                        # one-hot S[e, d] = (dloc[e] == d), bf16
                        S = svp.tile([P, Tm, P], BF16, tag="S")
                        nc.vector.tensor_tensor(
                            out=S[:, 0:TA_b, :],
                            in0=dlocA_sb[:, ac0 + ao:ac0 + ao + TA_b,
                                         None].to_broadcast([P, TA_b, P]),
                            in1=iota_sb[:, None, :].to_broadcast([P, TA_b, P]),
                            op=ALU.is_equal,
                        )
                        nc.vector.tensor_tensor(
                            out=S[:, TA_b:T_b, :],
                            in0=dlocB_sb[:, bc0 + cfg.boff[b]:
                                         bc0 + cfg.boff[b] + TB_b,
                                         None].to_broadcast([P, TB_b, P]),
                            in1=iota_sb[:, None, :].to_broadcast([P, TB_b, P]),
                            op=ALU.is_equal,
                        )

                        def gsl(i):
                            """G1 tile index of the block's i-th tile."""
                            return (ao + i) if i < TA_b else (bo + i - TA_b)

                        # elog[e,h] = a_dst[dloc[e],h] + a_src_gathered[e,h]
                        # accumulated in PSUM: St @ a_dst + I @ G_asrc
                        elps = adp.tile([P, Tm * 4], F32, tag="elps")
                        for t0 in range(0, T_b, 4):
                            tn = min(4, T_b - t0)
                            stps = sp.tile([P, 512], BF16, tag="stps")
                            for k in range(tn):
                                nc.tensor.transpose(
                                    out=stps[:, k * P:(k + 1) * P],
                                    in_=S[:, t0 + k, :], identity=ident_bf[:])
                            st4 = svp.tile([P, 4, P], BF16, tag="st4")
                            nc.vector.tensor_copy(out=st4[:, 0:tn, :],
                                                  in_=stps[:, 0:tn * P])
                            for k in range(tn):
                                t = t0 + k
                                nc.tensor.matmul(
                                    out=elps[:, t * 4:(t + 1) * 4],
                                    lhsT=st4[:, k, :],
                                    rhs=adst_sb[:, b * 4:(b + 1) * 4],
                                    start=True, stop=False)
                                nc.tensor.matmul(
                                    out=elps[:, t * 4:(t + 1) * 4],
                                    lhsT=ident_bf[:],
                                    rhs=G1[:, gsl(t), 128:132],
                                    start=False, stop=True)
                        # ex = exp(leaky_relu(elog))  on the ACT engine
                        # (Prelu honors alpha and shares the exp act table;
                        # Lrelu has a hardwired 0.01 slope)
                        lr = svp.tile([P, Tm * 4], F32, tag="lr")
                        nc.scalar.activation(out=lr[:, 0:T_b * 4],
                                             in_=elps[:, 0:T_b * 4],
                                             func=ACTF.Prelu, alpha=NEG_SLOPE)
                        ex = svp.tile([P, Tm, 4], F32, tag="ex")
                        nc.scalar.activation(
                            out=ex[:, 0:T_b, :],
                            in_=lr[:, 0:T_b * 4].rearrange(
                                "p (t f) -> p t f", f=4),
                            func=ACTF.Exp)
                        # V = [ex * xp | ex]  (bf16)
                        V = svp.tile([P, Tm, 132], BF16, tag="V")
                        for (u0, un, g0) in ((0, TA_b, ao), (TA_b, TB_b, bo)):
                            nc.vector.tensor_tensor(
                                out=V[:, u0:u0 + un, 0:128].rearrange(
                                    "p t (h c) -> p t h c", c=32),
                                in0=G1[:, g0:g0 + un, 0:128].rearrange(
                                    "p t (h c) -> p t h c", c=32),
                                in1=ex[:, u0:u0 + un, :,
                                       None].to_broadcast([P, un, 4, 32]),
                                op=ALU.mult,
                            )
                        nc.scalar.activation(out=V[:, 0:T_b, 128:132],
                                             in_=ex[:, 0:T_b, :],
                                             func=ACTF.Copy)
                        acc = ap.tile([P, 132], F32)
                        for t in range(T_b):
                            nc.tensor.matmul(out=acc[:], lhsT=S[:, t, :],
                                             rhs=V[:, t, :], start=(t == 0),
                                             stop=(t == T_b - 1))
                        # epilogue: divide, + skip, ELU
                        dn = ep.tile([P, 4], F32)
                        nc.vector.tensor_scalar_add(out=dn[:],
                                                    in0=acc[:, 128:132],
                                                    scalar1=1e-6)
                        rcp = ep.tile([P, 4], F32)
                        nc.vector.reciprocal(out=rcp[:], in_=dn[:])
                        y = ep.tile([P, 128], F32)
                        nc.vector.tensor_tensor(
                            out=y[:].rearrange("p (h c) -> p h c", c=32),
                            in0=acc[:, 0:128].rearrange("p (h c) -> p h c",
                                                        c=32),
                            in1=rcp[:, :, None].to_broadcast([P, 4, 32]),
                            op=ALU.mult,
                        )
                        y2 = ep.tile([P, 128], F32)
                        nc.vector.tensor_tensor(
                            out=y2[:], in0=y[:],
                            in1=skip_sb[:, b * P:(b + 1) * P], op=ALU.add)
                        # elu(v) = max(v,0) + exp(min(v,0)) - 1
                        mn = ep.tile([P, 128], F32)
                        nc.vector.tensor_scalar_min(out=mn[:], in0=y2[:],
                                                    scalar1=0.0)
                        e1 = ep.tile([P, 128], F32)
                        nc.scalar.activation(out=e1[:], in_=mn[:],
                                             func=ACTF.Exp)
                        mx = ep.tile([P, 128], F32)
                        nc.vector.tensor_scalar_max(out=mx[:], in0=y2[:],
                                                    scalar1=0.0)
                        yo = ep.tile([P, 128], F32)
                        nc.vector.scalar_tensor_tensor(
                            out=yo[:], in0=mx[:], scalar=-1.0, in1=e1[:],
                            op0=ALU.add, op1=ALU.add)
                        nc.scalar.dma_start(out=out[b * P:(b + 1) * P, :],
                                            in_=yo[:])

    nc.compile()
    return nc


# ---------------------------------------------------------------------------
# Public entry point.

_CACHE = {}


def _get_program(cfg):
    key = (cfg.N, cfg.E, cfg.NC, cfg.TAs, cfg.TBs)
    if key not in _CACHE:
        _CACHE[key] = build_program(cfg)
    return _CACHE[key]


def run_full(inputs, trace=False, **spmd_kwargs):
    cfg = Cfg()
    in_maps = make_inputs(cfg, **{k: np.asarray(v) for k, v in inputs.items()})
    nc = _get_program(cfg)
    res = run_bass_kernel_spmd(nc, in_maps, list(range(cfg.NC)), trace=trace,
                               **spmd_kwargs)
    outs = [res.results[c]["out"][:cfg.NLOC] for c in range(cfg.NC)]
    return np.concatenate(outs, axis=0).astype(np.float32), res


def kernel(x, edge_index, W, att_src, att_dst, bias, skip_W, skip_b):
    out, _ = run_full(dict(x=x, edge_index=edge_index, W=W, att_src=att_src,
                           att_dst=att_dst, bias=bias, skip_W=skip_W,
                           skip_b=skip_b))
    return out


# revision 15
# speedup vs baseline: 1.0183x; 1.0183x over previous
"""GAT residual block (nn_GATResBlock) on 8 Trainium2 NeuronCores.

Strategy
--------
- Shard destination nodes (and their incoming edges) across the 8 cores;
  each core owns a contiguous range of 6250 dst nodes.
- Host-side graph preprocessing (sanctioned by the sharding hint): sort each
  core's edges by dst block (128 dsts per block), build padded per-block edge
  lists and int16 gather-index arrays.
- Algebraic folds: a_src = x @ (W.T @ att_src-expanded) so the attention
  logits come out of the same projection matmul; segment-softmax max-trick is
  dropped (logits are bounded, softmax is shift invariant) and the softmax is
  normalized at the *node* level: agg = (sum ex*xp[src]) / (sum ex), so no
  per-edge alpha is ever materialized.
- Device per core: one replicated projection pass builds a DRAM node table
  T1[row] = [xp | a_src] (bf16); dst blocks are processed in groups of ~5 with
  ONE dma_gather per (group, table-half) to amortize the ~2.5us SWDGE call
  overhead.  Per block, a one-hot (edge,dst) selection matrix built with
  iota/is_equal routes a_dst to edges (tensor transpose + tiny matmul, with
  the gathered a_src accumulated into the same PSUM via an identity matmul),
  the LeakyReLU+exp run on the scalar (ACT) engine, and a PSUM-accumulated
  matmul reduces weighted messages + softmax denominators in one pass.
  Epilogue divides, adds the skip projection and applies ELU.
- int16 gather indices only span 32768 rows, so the node table is gathered by
  two calls per group: rows [0, 32768) ("A") and [32768, ...) ("B"); the host
  splits each block's edge list accordingly.  Pad slots gather table row 0
  (a_src = -60 so ex ~ 1e-5) and carry dloc = 500 so the one-hot zeroes them.
"""

import sys
import types

sys.path.insert(0, "/opt/trn_rl_repo")

import numpy as np

try:
    import ml_dtypes

    BF16_NP = ml_dtypes.bfloat16
except Exception:  # pragma: no cover
    BF16_NP = None


# ---------------------------------------------------------------------------
# NTFF profile hook (missing antenv.axon_hooks in this image). Needed only
# when tracing; harmless otherwise.
def _install_ntff_hook():
    if "antenv.axon_hooks" in sys.modules:
        return
    try:
        hooks = types.ModuleType("antenv.axon_hooks")
        _h = [None]
        hooks.set_axon_ntff_profile_hook = lambda h: _h.__setitem__(0, h)
        hooks.get_axon_ntff_profile_hook = lambda: _h[0]
        sys.modules["antenv.axon_hooks"] = hooks
        import antenv

        antenv.axon_hooks = hooks
        from trn_agent_boot.trn_boot import _ntff_profile_via_ctypes

        hooks.set_axon_ntff_profile_hook(
            _ntff_profile_via_ctypes("/opt/axon/libaxon_pjrt.so")
        )
    except Exception:
        pass


_install_ntff_hook()

from concourse import bacc, bass, mybir, tile  # noqa: E402
from concourse.bass_utils import run_bass_kernel_spmd  # noqa: E402

F32 = mybir.dt.float32
I16 = mybir.dt.int16
ALU = mybir.AluOpType
ACTF = mybir.ActivationFunctionType

P = 128
NEG_SLOPE = 0.2
PAD_ASRC = -60.0
PAD_DLOC = 500.0
GROUP = 3            # dst blocks per dma_gather call


class Cfg:
    def __init__(self, N=50000, IN=128, H=4, C=32, E=800000, NC=8, SPLIT=32768,
                 TAs=None, TBs=None):
        self.N, self.IN, self.H, self.C, self.E, self.NC = N, IN, H, C, E, NC
        self.HC = H * C
        assert self.HC == 128 and IN == 128
        assert N % NC == 0
        self.NLOC = N // NC                      # owned dst nodes per core
        self.NBLK = (self.NLOC + P - 1) // P     # dst blocks per core
        self.NLOCP = self.NBLK * P               # padded local nodes
        self.SPLIT = SPLIT                       # int16 A/B table split
        nrows = 1 + N + 1                        # PAD_A + nodes + PAD_B
        self.NR = ((nrows + P - 1) // P) * P     # node-table rows (padded)
        assert self.NR - SPLIT <= 32768
        self.PAD_B = N + 1                       # table row of the B pad
        self.ROWW = 256                  # T1 bf16 cols: xp(128)+a_src(4)+pad
        self.TAs, self.TBs = TAs, TBs            # per-block edge tiles (A/B)
        if TAs is not None:
            self._finish()

    def _finish(self):
        self.TAtot = sum(self.TAs)
        self.TBtot = sum(self.TBs)
        self.Tmax = max(a + b for a, b in zip(self.TAs, self.TBs))
        # block groups (gather-call granularity)
        self.groups = [list(range(g, min(g + GROUP, self.NBLK)))
                       for g in range(0, self.NBLK, GROUP)]
        self.gTA = [sum(self.TAs[b] for b in g) for g in self.groups]
        self.gTB = [sum(self.TBs[b] for b in g) for g in self.groups]
        self.gTAmax = max(self.gTA)
        self.gTBmax = max(self.gTB)
        # tile offset of block b's A (resp B) region within its group tile
        self.aoff, self.boff = {}, {}
        for gi, g in enumerate(self.groups):
            oa, ob = 0, 0
            for b in g:
                self.aoff[b] = oa
                self.boff[b] = ob
                oa += self.TAs[b]
                ob += self.TBs[b]
        # column offset of block b's A/B tiles in the concatenated idx arrays
        self.acol, self.bcol = {}, {}
        oa = ob = 0
        for b in range(self.NBLK):
            self.acol[b], self.bcol[b] = oa, ob
            oa += self.TAs[b]
            ob += self.TBs[b]


# ---------------------------------------------------------------------------
# Host-side preprocessing: edge partitioning + gather index construction.


def _wrap_idx(arr):
    """[K*128] edge-slot array -> [128, K*8] int16 'wrapped' index layout
    (index i lives at [i % 16, i // 16], replicated across the 8 groups)."""
    k16 = arr.reshape(-1, 16).T.astype(np.int16)  # [16, K*8]
    return np.tile(k16, (8, 1))                   # [128, K*8]


def preprocess(cfg, edge_index):
    """Build per-core gather index arrays from the (2, E) edge list."""
    src = np.asarray(edge_index[0], dtype=np.int64)
    dst = np.asarray(edge_index[1], dtype=np.int64)
    core = dst // cfg.NLOC
    dstl = dst - core * cfg.NLOC
    blk = dstl // P
    srow = src + 1                                # +1: table row 0 is PAD_A
    isB = (srow >= cfg.SPLIT).astype(np.int64)

    order = np.lexsort((srow, isB, blk, core))
    core_s, blk_s, isB_s = core[order], blk[order], isB[order]
    srow_s, dstl_s = srow[order], dstl[order]

    gid = ((core_s * cfg.NBLK) + blk_s) * 2 + isB_s
    ngroups = cfg.NC * cfg.NBLK * 2
    counts = np.bincount(gid, minlength=ngroups)
    starts = np.concatenate(([0], np.cumsum(counts)[:-1]))
    rank = np.arange(len(gid)) - starts[gid]

    cA = counts.reshape(cfg.NC, cfg.NBLK, 2)[:, :, 0]   # [NC, NBLK]
    cB = counts.reshape(cfg.NC, cfg.NBLK, 2)[:, :, 1]
    if cfg.TAs is None:
        cfg.TAs = tuple(max(1, int(-(-cA[:, b].max() // P)))
                        for b in range(cfg.NBLK))
        cfg.TBs = tuple(max(1, int(-(-cB[:, b].max() // P)))
                        for b in range(cfg.NBLK))
        cfg._finish()
    for b in range(cfg.NBLK):
        assert cA[:, b].max() <= cfg.TAs[b] * P
        assert cB[:, b].max() <= cfg.TBs[b] * P

    # per (core, block) padded slot arrays
    idxA = {}
    idxB = {}
    dlocA = {}
    dlocB = {}
    for c in range(cfg.NC):
        for b in range(cfg.NBLK):
            idxA[c, b] = np.zeros(cfg.TAs[b] * P, dtype=np.int64)
            idxB[c, b] = np.full(cfg.TBs[b] * P, cfg.PAD_B - cfg.SPLIT,
                                 dtype=np.int64)
            dlocA[c, b] = np.full(cfg.TAs[b] * P, PAD_DLOC, dtype=np.float32)
            dlocB[c, b] = np.full(cfg.TBs[b] * P, PAD_DLOC, dtype=np.float32)

    a = isB_s == 0
    b_ = ~a
    # vectorized fill
    for c in range(cfg.NC):
        for b in range(cfg.NBLK):
            selA = a & (core_s == c) & (blk_s == b)
            idxA[c, b][rank[selA]] = srow_s[selA]
            dlocA[c, b][rank[selA]] = dstl_s[selA] - b * P
            selB = b_ & (core_s == c) & (blk_s == b)
            idxB[c, b][rank[selB]] = srow_s[selB] - cfg.SPLIT
            dlocB[c, b][rank[selB]] = dstl_s[selB] - b * P

    per_core = []
    for c in range(cfg.NC):
        wA = np.concatenate([_wrap_idx(idxA[c, b]) for b in range(cfg.NBLK)],
                            axis=1)
        wB = np.concatenate([_wrap_idx(idxB[c, b]) for b in range(cfg.NBLK)],
                            axis=1)
        # dloc layout [128, tiles]: [p, tile_col + t] = slot (t, p)
        dA = np.concatenate(
            [dlocA[c, b].reshape(cfg.TAs[b], P).T for b in range(cfg.NBLK)],
            axis=1)
        dB = np.concatenate(
            [dlocB[c, b].reshape(cfg.TBs[b], P).T for b in range(cfg.NBLK)],
            axis=1)
        per_core.append(dict(idxA=np.ascontiguousarray(wA),
                             idxB=np.ascontiguousarray(wB),
                             dlocA=np.ascontiguousarray(dA),
                             dlocB=np.ascontiguousarray(dB)))
    return per_core


def make_weights(cfg, W, att_src, att_dst, bias, skip_W, skip_b):
    """Fold attention vectors into the projection weights."""
    H, C, IN = cfg.H, cfg.C, cfg.IN
    A_s = np.zeros((IN, H), dtype=np.float32)
    A_d = np.zeros((IN, H), dtype=np.float32)
    for h in range(H):
        # a_src[n,h] = sum_c xp[n,h*C+c]*att_src[h,c] = x @ (W[h*C:+C].T @ att)
        A_s[:, h] = W[h * C:(h + 1) * C, :].T @ att_src[0, h]
        A_d[:, h] = W[h * C:(h + 1) * C, :].T @ att_dst[0, h]
    Wcat = np.concatenate([W.T, A_s, A_d], axis=1).astype(np.float32)  # [IN,136]
    Wsk = np.concatenate([skip_W.T, A_d], axis=1).astype(np.float32)   # [IN,132]
    bias2 = np.tile((bias + skip_b).astype(np.float32)[None, :], (P, 1))
    return Wcat, Wsk, bias2


def make_inputs(cfg, x, edge_index, W, att_src, att_dst, bias, skip_W, skip_b):
    per_core_idx = preprocess(cfg, edge_index)
    Wcat, Wsk, bias2 = make_weights(cfg, W, att_src, att_dst, bias, skip_W,
                                    skip_b)
    xT = np.zeros((cfg.IN, cfg.NR), dtype=np.float32)
    xT[:, 1:1 + cfg.N] = np.asarray(x, dtype=np.float32).T
    xT = xT.astype(BF16_NP)
    iota = np.tile(np.arange(P, dtype=np.float32)[None, :], (P, 1))
    iotap = np.tile(np.arange(P, dtype=np.float32)[:, None], (1, P))
    negr = np.full((1, 4), PAD_ASRC, dtype=np.float32)

    in_maps = []
    for c in range(cfg.NC):
        xTl = np.zeros((cfg.IN, cfg.NLOCP), dtype=np.float32)
        xTl[:, :cfg.NLOC] = np.asarray(
            x[c * cfg.NLOC:(c + 1) * cfg.NLOC], dtype=np.float32).T
        m = dict(xT=xT, xTl=np.ascontiguousarray(xTl.astype(BF16_NP)),
                 Wcat=Wcat, Wsk=Wsk,
                 bias2=bias2, iota=iota, iotap=iotap, negr=negr,
                 **per_core_idx[c])
        in_maps.append(m)
    return in_maps


# ---------------------------------------------------------------------------
# Device program.


def build_program(cfg, debug_level=99):
    """Build the per-core SPMD Bass program."""
    nc = bacc.Bacc(None)
    NBLK, NR, ROWW = cfg.NBLK, cfg.NR, cfg.ROWW
    BF16 = mybir.dt.bfloat16

    xT = nc.declare_dram_parameter("xT", [cfg.IN, NR], BF16, isOutput=False)
    xTl = nc.declare_dram_parameter("xTl", [cfg.IN, cfg.NLOCP], BF16,
                                    isOutput=False)
    Wcat = nc.declare_dram_parameter("Wcat", [cfg.IN, 136], F32, isOutput=False)
    Wsk = nc.declare_dram_parameter("Wsk", [cfg.IN, 132], F32, isOutput=False)
    bias2 = nc.declare_dram_parameter("bias2", [P, 128], F32, isOutput=False)
    iota = nc.declare_dram_parameter("iota", [P, P], F32, isOutput=False)
    iotap = nc.declare_dram_parameter("iotap", [P, P], F32, isOutput=False)
    negr = nc.declare_dram_parameter("negr", [1, 4], F32, isOutput=False)
    idxA = nc.declare_dram_parameter("idxA", [P, cfg.TAtot * 8], I16,
                                     isOutput=False)
    idxB = nc.declare_dram_parameter("idxB", [P, cfg.TBtot * 8], I16,
                                     isOutput=False)
    dlocA = nc.declare_dram_parameter("dlocA", [P, cfg.TAtot], F32,
                                      isOutput=False)
    dlocB = nc.declare_dram_parameter("dlocB", [P, cfg.TBtot], F32,
                                      isOutput=False)
    out = nc.declare_dram_parameter("out", [cfg.NLOCP, 128], F32,
                                    isOutput=True)

    T1 = nc.dram_tensor("T1", [NR, ROWW], BF16)

    with tile.TileContext(nc) as tc:
        with (
            tc.tile_pool(name="const", bufs=1) as cpool,
            tc.tile_pool(name="main", bufs=3) as mp,
            tc.tile_pool(name="sv", bufs=3) as svp,
            tc.tile_pool(name="epi", bufs=4) as ep,
        ):
            # ---- constants ----
            iota_sb = cpool.tile([P, P], F32)
            nc.sync.dma_start(out=iota_sb[:], in_=iota[:])
            iotap_sb = cpool.tile([P, P], F32)
            nc.sync.dma_start(out=iotap_sb[:], in_=iotap[:])
            ident_bf = cpool.tile([P, P], BF16)
            nc.vector.tensor_tensor(out=ident_bf[:], in0=iota_sb[:],
                                    in1=iotap_sb[:], op=ALU.is_equal)
            wcat_sb = cpool.tile([P, 136], F32)
            nc.sync.dma_start(out=wcat_sb[:], in_=Wcat[:])
            wcat_bf = cpool.tile([P, 136], BF16)
            nc.vector.tensor_copy(out=wcat_bf[:], in_=wcat_sb[:])
            wsk_sb = cpool.tile([P, 132], F32)
            nc.sync.dma_start(out=wsk_sb[:], in_=Wsk[:])
            wsk_bf = cpool.tile([P, 132], BF16)
            nc.vector.tensor_copy(out=wsk_bf[:], in_=wsk_sb[:])
            bias_sb = cpool.tile([P, 128], F32)
            nc.sync.dma_start(out=bias_sb[:], in_=bias2[:])
            negr_sb = cpool.tile([1, 4], F32)
            nc.sync.dma_start(out=negr_sb[:], in_=negr[:])
            idxA_sb = cpool.tile([P, cfg.TAtot * 8], I16)
            nc.sync.dma_start(out=idxA_sb[:], in_=idxA[:])
            idxB_sb = cpool.tile([P, cfg.TBtot * 8], I16)
            nc.sync.dma_start(out=idxB_sb[:], in_=idxB[:])
            dlocA_sb = cpool.tile([P, cfg.TAtot], F32)
            nc.sync.dma_start(out=dlocA_sb[:], in_=dlocA[:])
            dlocB_sb = cpool.tile([P, cfg.TBtot], F32)
            nc.sync.dma_start(out=dlocB_sb[:], in_=dlocB[:])
            skip_sb = cpool.tile([P, NBLK * 128], F32)
            adst_sb = cpool.tile([P, NBLK * 4], BF16)

            # ---- phase 1: global node table T1 = [xp | a_src] (bf16) ----
            with (
                tc.tile_pool(name="prol", bufs=3) as prol,
                tc.tile_pool(name="pp", bufs=2, space="PSUM") as pp,
            ):
                CH = 16
                for i0 in range(0, NR // P, CH):
                    ch = min(CH, NR // P - i0)
                    xt = prol.tile([P, CH * P], BF16)
                    nc.sync.dma_start(
                        out=xt[:, 0:ch * P], in_=xT[:, i0 * P:(i0 + ch) * P])
                    st4 = prol.tile([P, CH, 132], BF16)
                    for k in range(ch):
                        ps = pp.tile([P, 136], F32)
                        nc.tensor.matmul(out=ps[:],
                                         lhsT=xt[:, k * P:(k + 1) * P],
                                         rhs=wcat_bf[:], start=True, stop=True)
                        if k % 2 == 0:
                            nc.scalar.activation(out=st4[:, k, :],
                                                 in_=ps[:, 0:132],
                                                 func=ACTF.Copy)
                        else:
                            nc.vector.tensor_copy(out=st4[:, k, :],
                                                  in_=ps[:, 0:132])
                    nc.scalar.dma_start(
                        out=T1[i0 * P:(i0 + ch) * P, 0:132].rearrange(
                            "(k p) c -> p k c", p=P),
                        in_=st4[:, 0:ch, :])
                # pad rows: a_src = PAD_ASRC so padded edges contribute ~0
                negr_bf = cpool.tile([1, 4], BF16)
                nc.vector.tensor_copy(out=negr_bf[:], in_=negr_sb[:])
                nc.sync.dma_start(out=T1[0:1, 128:132], in_=negr_bf[:])
                nc.sync.dma_start(out=T1[cfg.PAD_B:cfg.PAD_B + 1, 128:132],
                                  in_=negr_bf[:])

                # ---- phase 2: local skip projection + a_dst (SBUF) ----
                for j in range(NBLK):
                    xl = prol.tile([P, P], BF16)
                    nc.sync.dma_start(out=xl[:], in_=xTl[:, j * P:(j + 1) * P])
                    ps2 = pp.tile([P, 132], F32, tag="ps2")
                    nc.tensor.matmul(out=ps2[:], lhsT=xl[:], rhs=wsk_bf[:],
                                     start=True, stop=True)
                    nc.vector.tensor_tensor(out=skip_sb[:, j * P:(j + 1) * P],
                                            in0=ps2[:, 0:128], in1=bias_sb[:],
                                            op=ALU.add)
                    nc.scalar.activation(out=adst_sb[:, j * 4:(j + 1) * 4],
                                         in_=ps2[:, 128:132], func=ACTF.Copy)

            # ---- phase 3: per-dst-block edge processing ----
            Tm = cfg.Tmax
            with (
                tc.tile_pool(name="acc", bufs=2, space="PSUM") as ap,
                tc.tile_pool(name="stp", bufs=2, space="PSUM") as sp,
                tc.tile_pool(name="adp", bufs=2, space="PSUM") as adp,
            ):
                for gi, grp in enumerate(cfg.groups):
                    gTA, gTB = cfg.gTA[gi], cfg.gTB[gi]
                    ac0 = cfg.acol[grp[0]]
                    bc0 = cfg.bcol[grp[0]]
                    GA = mp.tile([P, cfg.gTAmax, ROWW], BF16, tag="GA")
                    GB = mp.tile([P, cfg.gTBmax, ROWW], BF16, tag="GB")
                    nc.gpsimd.dma_gather(
                        out_ap=GA[:, 0:gTA, :],
                        in_ap=T1[0:cfg.SPLIT, :],
                        idxs_ap=idxA_sb[:, ac0 * 8:(ac0 + gTA) * 8],
                        num_idxs=gTA * P,
                        num_idxs_reg=gTA * P,
                        elem_size=ROWW,
                        single_packet=False,
                    )
                    nc.gpsimd.dma_gather(
                        out_ap=GB[:, 0:gTB, :],
                        in_ap=T1[cfg.SPLIT:, :],
                        idxs_ap=idxB_sb[:, bc0 * 8:(bc0 + gTB) * 8],
                        num_idxs=gTB * P,
                        num_idxs_reg=gTB * P,
                        elem_size=ROWW,
                        single_packet=False,
                    )

                    for b in grp:
                        TA_b, TB_b = cfg.TAs[b], cfg.TBs[b]
                        T_b = TA_b + TB_b
                        ao = cfg.aoff[b]              # tiles into GA
                        bo = cfg.boff[b]              # tiles into GB

                        # one-hot S[e, d] = (dloc[e] == d), bf16
                        S = svp.tile([P, Tm, P], BF16, tag="S")
                        nc.vector.tensor_tensor(
                            out=S[:, 0:TA_b, :],
                            in0=dlocA_sb[:, ac0 + ao:ac0 + ao + TA_b,
                                         None].to_broadcast([P, TA_b, P]),
                            in1=iota_sb[:, None, :].to_broadcast([P, TA_b, P]),
                            op=ALU.is_equal,
                        )
                        nc.vector.tensor_tensor(
                            out=S[:, TA_b:T_b, :],
                            in0=dlocB_sb[:, bc0 + cfg.boff[b]:
                                         bc0 + cfg.boff[b] + TB_b,
                                         None].to_broadcast([P, TB_b, P]),
                            in1=iota_sb[:, None, :].to_broadcast([P, TB_b, P]),
                            op=ALU.is_equal,
                        )

                        def gsl(i):
                            """(tensor, tile index) of the block's i-th tile."""
                            return ((GA, ao + i) if i < TA_b
                                    else (GB, bo + i - TA_b))

                        # elog[e,h] = a_dst[dloc[e],h] + a_src_gathered[e,h]
                        # accumulated in PSUM: St @ a_dst + I @ G_asrc
                        elps = adp.tile([P, Tm * 4], F32, tag="elps")
                        for t0 in range(0, T_b, 4):
                            tn = min(4, T_b - t0)
                            stps = sp.tile([P, 512], BF16, tag="stps")
                            for k in range(tn):
                                nc.tensor.transpose(
                                    out=stps[:, k * P:(k + 1) * P],
                                    in_=S[:, t0 + k, :], identity=ident_bf[:])
                            st4 = svp.tile([P, 4, P], BF16, tag="st4")
                            nc.vector.tensor_copy(out=st4[:, 0:tn, :],
                                                  in_=stps[:, 0:tn * P])
                            for k in range(tn):
                                t = t0 + k
                                gt, gti = gsl(t)
                                nc.tensor.matmul(
                                    out=elps[:, t * 4:(t + 1) * 4],
                                    lhsT=st4[:, k, :],
                                    rhs=adst_sb[:, b * 4:(b + 1) * 4],
                                    start=True, stop=False)
                                nc.tensor.matmul(
                                    out=elps[:, t * 4:(t + 1) * 4],
                                    lhsT=ident_bf[:],
                                    rhs=gt[:, gti, 128:132],
                                    start=False, stop=True)
                        # ex = exp(leaky_relu(elog))  on the ACT engine
                        # (Prelu honors alpha and shares the exp act table;
                        # Lrelu has a hardwired 0.01 slope)
                        lr = svp.tile([P, Tm * 4], F32, tag="lr")
                        nc.scalar.activation(out=lr[:, 0:T_b * 4],
                                             in_=elps[:, 0:T_b * 4],
                                             func=ACTF.Prelu, alpha=NEG_SLOPE)
                        ex = svp.tile([P, Tm, 4], F32, tag="ex")
                        nc.scalar.activation(
                            out=ex[:, 0:T_b, :],
                            in_=lr[:, 0:T_b * 4].rearrange(
                                "p (t f) -> p t f", f=4),
                            func=ACTF.Exp)
                        # V = [ex * xp | ex]  (bf16)
                        V = svp.tile([P, Tm, 132], BF16, tag="V")
                        for (u0, un, gt, g0) in ((0, TA_b, GA, ao),
                                                 (TA_b, TB_b, GB, bo)):
                            nc.vector.tensor_tensor(
                                out=V[:, u0:u0 + un, 0:128].rearrange(
                                    "p t (h c) -> p t h c", c=32),
                                in0=gt[:, g0:g0 + un, 0:128].rearrange(
                                    "p t (h c) -> p t h c", c=32),
                                in1=ex[:, u0:u0 + un, :,
                                       None].to_broadcast([P, un, 4, 32]),
                                op=ALU.mult,
                            )
                        nc.scalar.activation(out=V[:, 0:T_b, 128:132],
                                             in_=ex[:, 0:T_b, :],
                                             func=ACTF.Copy)
                        acc = ap.tile([P, 132], F32)
                        for t in range(T_b):
                            nc.tensor.matmul(out=acc[:], lhsT=S[:, t, :],
                                             rhs=V[:, t, :], start=(t == 0),
                                             stop=(t == T_b - 1))
                        # epilogue: divide, + skip, ELU
                        dn = ep.tile([P, 4], F32)
                        nc.vector.tensor_scalar_add(out=dn[:],
                                                    in0=acc[:, 128:132],
                                                    scalar1=1e-6)
                        rcp = ep.tile([P, 4], F32)
                        nc.vector.reciprocal(out=rcp[:], in_=dn[:])
                        y = ep.tile([P, 128], F32)
                        nc.vector.tensor_tensor(
                            out=y[:].rearrange("p (h c) -> p h c", c=32),
                            in0=acc[:, 0:128].rearrange("p (h c) -> p h c",
                                                        c=32),
                            in1=rcp[:, :, None].to_broadcast([P, 4, 32]),
                            op=ALU.mult,
                        )
                        y2 = ep.tile([P, 128], F32)
                        nc.vector.tensor_tensor(
                            out=y2[:], in0=y[:],
                            in1=skip_sb[:, b * P:(b + 1) * P], op=ALU.add)
                        # elu(v) = max(v,0) + exp(min(v,0)) - 1
                        mn = ep.tile([P, 128], F32)
                        nc.vector.tensor_scalar_min(out=mn[:], in0=y2[:],
                                                    scalar1=0.0)
                        e1 = ep.tile([P, 128], F32)
                        nc.scalar.activation(out=e1[:], in_=mn[:],
                                             func=ACTF.Exp)
                        mx = ep.tile([P, 128], F32)
                        nc.vector.tensor_scalar_max(out=mx[:], in0=y2[:],
                                                    scalar1=0.0)
                        yo = ep.tile([P, 128], F32)
                        nc.vector.scalar_tensor_tensor(
                            out=yo[:], in0=mx[:], scalar=-1.0, in1=e1[:],
                            op0=ALU.add, op1=ALU.add)
                        nc.scalar.dma_start(out=out[b * P:(b + 1) * P, :],
                                            in_=yo[:])

    nc.compile()
    return nc


# ---------------------------------------------------------------------------
# Public entry point.

_CACHE = {}


def _get_program(cfg):
    key = (cfg.N, cfg.E, cfg.NC, cfg.TAs, cfg.TBs)
    if key not in _CACHE:
        _CACHE[key] = build_program(cfg)
    return _CACHE[key]


def run_full(inputs, trace=False, **spmd_kwargs):
    cfg = Cfg()
    in_maps = make_inputs(cfg, **{k: np.asarray(v) for k, v in inputs.items()})
    nc = _get_program(cfg)
    res = run_bass_kernel_spmd(nc, in_maps, list(range(cfg.NC)), trace=trace,
                               **spmd_kwargs)
    outs = [res.results[c]["out"][:cfg.NLOC] for c in range(cfg.NC)]
    return np.concatenate(outs, axis=0).astype(np.float32), res


def kernel(x, edge_index, W, att_src, att_dst, bias, skip_W, skip_b):
    out, _ = run_full(dict(x=x, edge_index=edge_index, W=W, att_src=att_src,
                           att_dst=att_dst, bias=bias, skip_W=skip_W,
                           skip_b=skip_b))
    return out
